# revision 23
# baseline (speedup 1.0000x reference)
"""DeChunk EMA-scan kernel for Trainium2 (Bass/Tile), 8 NeuronCores. V2.

Problem: out[b,t,:] = p_t * x_t + (1-p_t) * out[b,t-1,:], where
x_t = hidden[b, idx_t, :], idx = cumsum(boundary_mask)-1,
p = clip(boundary_prob[...,1], EPS, 1-EPS) with p[:,0]=1.

Sharding: pure data parallel. core c handles batch b=c//2 and channel half
dh=c%2 (512 of 1024 channels). No cross-core communication.

V2 design (vs V1 baseline in kernel_baseline.py):
  - All data-dependent-but-cheap prep is HOST-side: gather indices (int16
    wrapped layout), tile-local log-decay cumsum S (rotated + flattened),
    exp(S) carry-weight rows, per-partition exp biases, level-2 carry
    weights. Device does no cumsums -> gathers start ~8us earlier.
  - Per-tile big matmuls are PURE LOCAL scans (no carry input) and fully
    independent -> PE streams back-to-back and ramps out of the low p-state.
  - Tile carries via a LEVEL-2 blocked scan: per-tile last-row r_k collected
    at the legal partition starts 0/32/64/96 of an R4 tile (4 tiles/block);
    one K=128 matmul + one K=1 carry-in matmul per block produce the 4
    carries (block-rotated so the block carry lands at partition 0). Serial
    chain cost ~0.3us/tile instead of ~0.9 (DVE recurrence) or ~1.4 (fold).
  - Carry applied per tile by a K=1 fixup matmul accumulating into the same
    PSUM bank (exp(S) row (x) carry).
  - S-broadcast matmul in PLAIN f32 (f32r there cost 2.7e-2 rel err: S~90
    loses mantissa and exp amplifies it). Data matmuls stay f32r.
  - Output stores on the Sync HWDGE queue; GpSimd runs only the gather.
"""

import sys

for _p in ("/opt/trn_rl_repo", "/root/.axon_site/_ro/trn_rl_repo"):
    if _p not in sys.path:
        sys.path.insert(0, _p)

import numpy as np
from contextlib import ExitStack

import concourse.bass as bass
import concourse.tile as tile
from concourse import bacc, mybir
from concourse._compat import with_exitstack

B, L, D = 4, 8192, 1024
N_CORES = 8
DC = D // 2  # channels per core
T = 128  # scan tile length
BLK = 4  # tiles per level-2 carry block
SCH = 4  # tiles per S-broadcast chunk
STG = 4  # output tiles per store DMA
EPS = 1e-4
STORE_SYNC = False
F32 = mybir.dt.float32
F32R = mybir.dt.float32r
I16 = mybir.dt.int16
ALU = mybir.AluOpType
ACTF = mybir.ActivationFunctionType


@with_exitstack
def _dechunk_v2(
    ctx: ExitStack,
    tc: "tile.TileContext",
    out_ap: bass.AP,
    hid_ap: bass.AP,
    idx16_ap: bass.AP,
    sflat_ap: bass.AP,
    arow_ap: bass.AP,
    biasv_ap: bass.AP,
    mbias_ap: bass.AP,
    ones_ap: bass.AP,
    wlv2_ap: bass.AP,
    a4rot_ap: bass.AP,
    zr4_ap: bass.AP,
    arb_ap: bass.AP,
    sbc2_ap: bass.AP,
    Lk: int,
    Dk: int,
    carry1: bool = True,
):
    nc = tc.nc
    nt = Lk // T
    ns = Lk // 16
    assert nt % BLK == 0 and nt % SCH == 0 and nt % STG == 0
    nb = nt // BLK

    const = ctx.enter_context(tc.tile_pool(name="const", bufs=1))
    gat_pool = ctx.enter_context(tc.tile_pool(name="gat", bufs=5))
    lhs_pool = ctx.enter_context(tc.tile_pool(name="lhs", bufs=4))
    r4_pool = ctx.enter_context(tc.tile_pool(name="r4", bufs=2))
    c4sb_pool = ctx.enter_context(tc.tile_pool(name="c4sb", bufs=3))
    cflat_pool = ctx.enter_context(tc.tile_pool(name="cflat", bufs=2))
    outsb_pool = ctx.enter_context(tc.tile_pool(name="outsb", bufs=2))
    psum_ops = ctx.enter_context(
        tc.tile_pool(name="psum_ops", bufs=8 if carry1 else 6, space="PSUM")
    )
    psum_scol = None
    if not carry1:
        psum_scol = ctx.enter_context(
            tc.tile_pool(name="psum_scol", bufs=2, space="PSUM")
        )

    # ---- constant / host-precomputed loads. idx16 first (gathers wait on it);
    # the lhsT-path constants go on the scalar HWDGE queue in parallel.
    idx16 = const.tile([T, ns], I16)
    nc.sync.dma_start(out=idx16[:], in_=idx16_ap)
    arb = sbc2 = None
    ones_r = sflat = cmb = biasv = arow = wlv2 = a4rot = None
    if carry1:
        # sbc2: host-prebuilt EXP input (S broadcast + causal mask + bias);
        # arb: fixup lhsT bank (row 32*(k%4) of column-block k = exp(S)-row
        # k, zeros elsewhere) -> fixups are K=128 matmuls, same shape as the
        # big matmuls (shape-uniform PE work keeps the clock ramped).
        # Interleave the two big loads in pieces so early tiles start fast.
        sbc2 = const.tile([T, Lk], F32)
        arb = const.tile([T, Lk], F32R)
        npc = 8
        for i in range(npc):
            s0, s1 = i * (Lk // npc), (i + 1) * (Lk // npc)
            nc.scalar.dma_start(out=sbc2[:, s0:s1], in_=sbc2_ap[:, s0:s1])
            nc.scalar.dma_start(out=arb[:, s0:s1], in_=arb_ap[:, s0:s1])
    else:
        ones_r = const.tile([2, T], F32R)
        nc.scalar.dma_start(out=ones_r[:], in_=ones_ap)
        sflat = const.tile([2, Lk], F32R)
        nc.scalar.dma_start(out=sflat[:], in_=sflat_ap)
        cmb = const.tile([T, T], F32)
        nc.scalar.dma_start(out=cmb[:], in_=mbias_ap)
        biasv = const.tile([T, nt], F32)
        nc.scalar.dma_start(out=biasv[:], in_=biasv_ap)
        arow = const.tile([1, Lk], F32R)
        nc.scalar.dma_start(out=arow[:], in_=arow_ap)
        wlv2 = const.tile([T, nt], F32R)
        nc.scalar.dma_start(out=wlv2[:], in_=wlv2_ap)
        a4rot = const.tile([1, nt], F32R)
        nc.scalar.dma_start(out=a4rot[:], in_=a4rot_ap)

    # R4 collection tiles: rows 32j hold r_{4b+j}; all other partitions must
    # be exactly 0.0 (they stream through the level-2 matmul with zero
    # weights, and garbage NaN/Inf would poison 0*x).
    r4_tiles = [
        r4_pool.tile([T, Dk], F32R, tag="r4", name=f"r4_{i}") for i in range(2)
    ]
    for t4 in r4_tiles:
        nc.scalar.dma_start(out=t4[:], in_=zr4_ap)

    # ---- gather emission (SWDGE q1), just-in-time with small leading chunks
    chunk_tiles = []
    for sz in (1, 1, 2, 4):
        if sum(chunk_tiles) + sz <= nt:
            chunk_tiles.append(sz)
    while sum(chunk_tiles) < nt:
        chunk_tiles.append(min(4, nt - sum(chunk_tiles)))
    chunk_start = [sum(chunk_tiles[:i]) for i in range(len(chunk_tiles))]
    tile2chunk = {}
    for c, (st, sz) in enumerate(zip(chunk_start, chunk_tiles)):
        for jj in range(sz):
            tile2chunk[st + jj] = c

    gat_tiles = {}

    def emit_gather(c):
        if c >= len(chunk_tiles):
            return
        n_idx = chunk_tiles[c] * T
        g_t = gat_pool.tile([T, chunk_tiles[c] * Dk], F32R, tag="gat", name=f"gat_{c}")
        g3 = g_t[:].rearrange("p (j d) -> p j d", d=Dk)
        s0 = chunk_start[c] * T // 16
        nc.gpsimd.dma_gather(
            out_ap=g3,
            in_ap=hid_ap.bitcast(F32R),
            idxs_ap=idx16[:, s0 : s0 + n_idx // 16],
            num_idxs=n_idx,
            num_idxs_reg=n_idx,
            elem_size=Dk,
            queue_num=1 if nc.num_swdge_queues > 1 else 0,
        )
        gat_tiles[c] = g_t

    GA_TILES = 16
    emitted_chunks = 0
    emitted_tiles = 0

    def advance_gathers(k):
        nonlocal emitted_chunks, emitted_tiles
        while emitted_tiles < min(k + GA_TILES, nt):
            emit_gather(emitted_chunks)
            emitted_tiles += chunk_tiles[emitted_chunks]
            emitted_chunks += 1

    advance_gathers(0)

    # ---- main loop
    # Output rows ROTATED by one: out partition t' holds position (t'-1)%T,
    # so each tile's last position (the carry row r_k) is at partition 0.
    ops_tiles = {}
    c4sb_tiles = {}
    cflat_tiles = {}
    osb_tiles = {}

    scol_tiles = {}

    def emit_scol(c):
        # S-broadcast chunk for tiles 4c..4c+3, emitted ~2 tiles early so the
        # mask-add + EXP chain never stalls the first big matmul of a block
        if c >= nt // SCH:
            return
        tmp_ps = psum_scol.tile([T, SCH * T], F32, tag="scol")
        nc.tensor.matmul(
            tmp_ps[:],
            lhsT=ones_r[:],
            rhs=sflat[:, c * SCH * T : (c + 1) * SCH * T],
            start=True,
            stop=True,
        )
        scol_tiles[c] = tmp_ps

    def emit_front(k):
        # lhsT build + pure-local big matmul for tile k
        lhsT_k = lhs_pool.tile([T, T], F32R, tag="lhsT")
        if carry1:
            nc.scalar.activation(
                lhsT_k[:], sbc2[:, k * T : (k + 1) * T], ACTF.Exp
            )
        else:
            tmp_ps = scol_tiles[k // SCH]
            j = k % SCH
            nc.vector.tensor_tensor(
                out=tmp_ps[:, j * T : (j + 1) * T],
                in0=tmp_ps[:, j * T : (j + 1) * T],
                in1=cmb[:],
                op=ALU.add,
            )
            nc.scalar.activation(
                lhsT_k[:],
                tmp_ps[:, j * T : (j + 1) * T],
                ACTF.Exp,
                bias=biasv[:, k : k + 1],
                scale=1.0,
            )
            if j == SCH - 1:
                scol_tiles.pop(k // SCH)
        ops = psum_ops.tile([T, Dk], F32, tag="ops")
        cg = tile2chunk[k]
        rhs = gat_tiles[cg][:].rearrange("p (j d) -> p j d", d=Dk)[
            :, k - chunk_start[cg], :
        ]
        nc.tensor.matmul(ops[:], lhsT=lhsT_k[:], rhs=rhs, start=True, stop=True)
        ops_tiles[k] = ops
        # collect r_k = local scan last row (rotated to partition 0) into the
        # SHIFTED window tile: consumer block i = (k+1)//BLK reads r rows of
        # tiles 4i-1..4i+2 at partitions 32*((k+1)%BLK)
        if k < nt - 1:
            i = (k + 1) // BLK
            jb = (k + 1) % BLK
            r4 = r4_tiles[i % 2]
            if k % 2 == 0:
                nc.scalar.copy(r4[32 * jb : 32 * jb + 1, :], ops[0:1, :])
            else:
                nc.vector.tensor_copy(r4[32 * jb : 32 * jb + 1, :], ops[0:1, :])

    def emit_block_carries(b):
        # level-2 over the SHIFTED window (tiles 4b-1..4b+2): produces the
        # carries needed by fixups of tiles 4b..4b+3 one tile early, so the
        # c4sb-copy + cflat-DMA latency is fully hidden. Block-rotated:
        # C4 row 0 = c_{4b+2}; rows 1..3 = c_{4b-1}, c_{4b}, c_{4b+1}.
        r4 = r4_tiles[b % 2]
        c4_ps = psum_scol.tile([BLK, Dk], F32, tag="scol")
        nc.tensor.matmul(
            c4_ps[:],
            lhsT=wlv2[:, b * BLK : (b + 1) * BLK],
            rhs=r4[:],
            start=True,
            stop=(b == 0),
        )
        if b > 0:
            nc.tensor.matmul(
                c4_ps[:],
                lhsT=a4rot[0:1, b * BLK : (b + 1) * BLK],
                rhs=c4sb_tiles[b - 1][0:1, :],
                start=False,
                stop=True,
            )
        c4sb = c4sb_pool.tile([BLK, Dk], F32R, tag="c4sb", name=f"c4sb_{b}")
        nc.vector.tensor_copy(c4sb[:], c4_ps[:])
        c4sb_tiles[b] = c4sb
        # rows 1..3 (c_{4b}..c_{4b+2}) flattened to partition 0 for the
        # within-block fixup matmul rhs reads
        cflat = cflat_pool.tile([1, (BLK - 1) * Dk], F32R, tag="cflat", name=f"cf_{b}")
        nc.sync.dma_start(
            out=cflat[:].rearrange("p (j d) -> p j d", d=Dk), in_=c4sb[1:BLK, :]
        )
        cflat_tiles[b] = cflat
        if b >= 2:
            c4sb_tiles.pop(b - 2)
            cflat_tiles.pop(b - 2)

    def emit_fixup(k):
        # ops_k += exp(S)-row (x) c_{k-1}, accumulated into the same bank
        if k == 0:
            return
        b, jb = k // BLK, k % BLK
        if carry1:
            # c_{k-1} = r_{k-1} exactly (inter-tile decay underflows f32).
            # K=128 form: lhsT = ARB column block (zeros except row 32*jb
            # which holds exp(S)-row k); rhs = whole r4 window tile.
            nc.tensor.matmul(
                ops_tiles[k][:],
                lhsT=arb[:, k * T : (k + 1) * T],
                rhs=r4_tiles[(k // BLK) % 2][:],
                start=False,
                stop=True,
                skip_group_check=True,
            )
            return
        elif jb == BLK - 1:
            c_prev = c4sb_tiles[b][0:1, :]
        else:
            c_prev = cflat_tiles[b][0:1, jb * Dk : (jb + 1) * Dk]
        nc.tensor.matmul(
            ops_tiles[k][:],
            lhsT=arow[0:1, k * T : (k + 1) * T],
            rhs=c_prev,
            start=False,
            stop=True,
            skip_group_check=True,
        )

    def emit_back(k):
        # final PSUM -> SBUF staging + batched store (sync HWDGE)
        ops = ops_tiles.pop(k)
        if k % STG == 0:
            osb_tiles[k // STG] = outsb_pool.tile(
                [T, STG * Dk], F32, tag="osb", name=f"osb_{k // STG}"
            )
        osb = osb_tiles[k // STG]
        dst = osb[:, (k % STG) * Dk : (k % STG + 1) * Dk]
        if k % 2 == 0:
            nc.scalar.copy(dst, ops[:])
        else:
            nc.vector.tensor_copy(dst, ops[:])
        if k % STG == STG - 1:
            g0 = k - (STG - 1)
            store_eng = nc.sync if STORE_SYNC else nc.gpsimd
            store_eng.dma_start(
                out=out_ap.rearrange("p (k d) -> p k d", d=Dk)[:, g0 : g0 + STG, :],
                in_=osb_tiles.pop(k // STG)[:].rearrange("p (k d) -> p k d", d=Dk),
            )

    # Fixups/stores for block b-1 are emitted while block b's big matmuls
    # stream, so the PE never waits on the c4sb copy + cflat DMA latency
    # (gapless PE keeps the p-state ramp alive).
    if not carry1:
        emit_scol(0)
    if carry1:
        # fixups/stores for block b-1 batched at j==1 of block b: all their
        # r-copies are >=2 tiles old (no stalls) and the matmul stream stays
        # shape-uniform for the PE clock ramp
        for k in range(nt):
            advance_gathers(k)
            b, jb = k // BLK, k % BLK
            if jb == 1 and b >= 1:
                for kk in range((b - 1) * BLK, b * BLK):
                    emit_fixup(kk)
                for kk in range((b - 1) * BLK, b * BLK):
                    emit_back(kk)
            emit_front(k)
        for kk in range(nt - BLK, nt):
            emit_fixup(kk)
        for kk in range(nt - BLK, nt):
            emit_back(kk)
    else:
        for k in range(nt):
            advance_gathers(k)
            b, jb = k // BLK, k % BLK
            if jb == 1 and b >= 1:
                for kk in range((b - 1) * BLK, b * BLK):
                    emit_fixup(kk)
                for kk in range((b - 1) * BLK, b * BLK):
                    emit_back(kk)
            if jb == 2:
                emit_scol(b + 1)
            emit_front(k)
            if jb == BLK - 1:
                emit_block_carries(b)
        for kk in range((nb - 1) * BLK, nt):
            emit_fixup(kk)
        for kk in range((nb - 1) * BLK, nt):
            emit_back(kk)


def _host_constants():
    s = np.arange(T)[:, None]
    t = np.arange(T)[None, :]
    t_rot = (t - 1) % T
    mbias = np.where(s <= t_rot, 0.0, -3e38).astype(np.float32)
    ones = np.ones((1, T), dtype=np.float32)
    return mbias, ones


def build_nc(Lk=L, Dk=DC, carry1=True):
    nt = Lk // T
    nc = bacc.Bacc(
        "TRN2",
        target_bir_lowering=False,
        debug=False,
        enable_asserts=False,
        num_swdge_queues=2,
    )
    hid = nc.dram_tensor("hid", [Lk, Dk], F32, kind="ExternalInput").ap()
    idx16 = nc.dram_tensor("idx16", [T, Lk // 16], I16, kind="ExternalInput").ap()
    sflat = nc.dram_tensor("sflat", [2, Lk], F32R, kind="ExternalInput").ap()
    arow = nc.dram_tensor("arow", [1, Lk], F32R, kind="ExternalInput").ap()
    biasv = nc.dram_tensor("biasv", [T, nt], F32, kind="ExternalInput").ap()
    mbias = nc.dram_tensor("mbias", [T, T], F32, kind="ExternalInput").ap()
    ones = nc.dram_tensor("ones", [2, T], F32R, kind="ExternalInput").ap()
    wlv2 = nc.dram_tensor("wlv2", [T, nt], F32R, kind="ExternalInput").ap()
    a4rot = nc.dram_tensor("a4rot", [1, nt], F32R, kind="ExternalInput").ap()
    zr4 = nc.dram_tensor("zr4", [T, Dk], F32R, kind="ExternalInput").ap()
    arb = nc.dram_tensor("arb", [T, Lk], F32R, kind="ExternalInput").ap()
    sbc2 = nc.dram_tensor("sbc2", [T, Lk], F32, kind="ExternalInput").ap()
    # raw partition-major layout: out[p, k*Dk + d] = y[k*T + (p-1)%T, d]
    out = nc.dram_tensor("out", [T, nt * Dk], F32, kind="ExternalOutput").ap()
    with tile.TileContext(nc) as tc:
        _dechunk_v2(
            tc, out, hid, idx16, sflat, arow, biasv, mbias, ones, wlv2, a4rot, zr4, arb, sbc2, Lk, Dk,
            carry1=carry1,
        )
    nc.compile()
    return nc


def unpermute_out(raw, Lk=L, Dk=DC):
    """raw (T, nt*Dk) partition-major rotated -> (Lk, Dk) sequence order."""
    nt = Lk // T
    raw = raw.reshape(T, nt, Dk)
    rr = raw[(np.arange(T) + 1) % T]  # rr[q, k] = y[k*T + q]
    return np.ascontiguousarray(rr.transpose(1, 0, 2).reshape(Lk, Dk))


def make_core_inputs(hid_c, p_c, m_c, Lk=L):
    """Host-side prep. hid_c (Lk, Dk) f32; p_c (Lk,) raw probs; m_c (Lk,) mask."""
    nt = Lk // T
    idx = np.cumsum(np.asarray(m_c, dtype=np.int64)) - 1
    idxw = idx.astype(np.int16).reshape(Lk // 16, 16).T  # wrapped [16, ns]
    idx16 = np.ascontiguousarray(np.tile(idxw, (8, 1)))  # [128, ns]

    p = np.clip(np.asarray(p_c, dtype=np.float64), EPS, 1.0 - EPS)
    p[0] = 1.0
    a = 1.0 - p
    a[0] = 1.0  # cancels in all weights; position 0 has no carry
    loga = np.log(a).reshape(nt, T)
    S = np.cumsum(loga, axis=1)  # tile-local inclusive cumsum [nt, T]
    biasv = (np.log(p).reshape(nt, T) - S).T.astype(np.float32)  # [T, nt]
    rot = (np.arange(T) - 1) % T
    srot = S[:, rot].reshape(Lk)  # srot[k*T+t'] = S[k, (t'-1)%T]
    arow = np.exp(srot).astype(np.float32).reshape(1, Lk)
    # coarse/fine split so the K=2 f32r broadcast matmul reconstructs S
    # exactly: coarse keeps 10 mantissa bits (f32r-exact for any >=10-bit
    # format); fine = S - coarse is ~2^-10 smaller, its own rounding error
    # is negligible after exp().
    sc = np.ascontiguousarray(srot.astype(np.float32))
    sc_i = sc.view(np.uint32)
    sc_i &= np.uint32(0xFFFFE000)  # keep 10 explicit mantissa bits
    fine = (srot - sc.astype(np.float64)).astype(np.float32)
    sflat = np.stack([sc, fine]).reshape(2, Lk)

    # level-2 carry weights: c_k = A_k c_{k-1} + r_k, A_k = exp(S[k, -1]).
    # Block b covers tiles k0..k0+3; C4 row j' holds c_{k0+(j'+3)%4}.
    A = np.exp(S[:, -1])  # [nt]
    # Shifted level-2 windows: block b combines r of tiles 4b-1..4b+2 (slot m
    # holds tile 4b-1+m; slot 0 of block 0 is the zero r_{-1} row). Rotated
    # columns: j'=0 -> c_{4b+2}, j'=1..3 -> c_{4b-1..4b+1}.
    wlv2 = np.zeros((T, nt), dtype=np.float32)
    a4rot = np.zeros((1, nt), dtype=np.float32)
    for b in range(nt // BLK):
        for jp in range(BLK):
            jj = (jp + BLK - 1) % BLK
            acc = 1.0
            for m in range(jj, -1, -1):
                w = b * BLK - 1 + m  # global tile index of window slot m
                if w >= 0:
                    # weight of r_{w} into c_{4b-1+jj}
                    wlv2[32 * m, b * BLK + jp] = np.float32(acc)
                    acc *= A[w]
                else:
                    acc = 0.0  # r_{-1}: no contribution, no carry-in
            a4rot[0, b * BLK + jp] = np.float32(acc)
    mbias, ones = _host_constants()
    ones = np.ones((2, T), dtype=np.float32)
    arb = np.zeros((T, Lk), dtype=np.float32)
    arow_f = arow.reshape(nt, T)
    for k in range(1, nt):
        arb[32 * (k % BLK), k * T : (k + 1) * T] = arow_f[k]
    # sbc2[s, k*T+t'] = S[k,(t'-1)%T] + causal-mask(s,t') + (logp - S)[k,s]:
    # the full EXP input precomputed in f64, one Scal op per tile on device
    srot_kt = S[:, rot]  # [nt, T]
    bias_ks = np.log(p).reshape(nt, T) - S  # [nt, T]
    sbc2 = (
        srot_kt[None, :, :]
        + mbias.astype(np.float64)[:, None, :]
        + bias_ks.T[:, :, None]
    )
    sbc2 = np.ascontiguousarray(
        np.maximum(sbc2, -3e38).astype(np.float32).reshape(T, Lk)
    )
    return {
        "hid": np.ascontiguousarray(hid_c, dtype=np.float32),
        "idx16": idx16,
        "sflat": np.ascontiguousarray(sflat),
        "arow": np.ascontiguousarray(arow),
        "biasv": np.ascontiguousarray(biasv),
        "mbias": mbias,
        "ones": ones,
        "wlv2": wlv2,
        "a4rot": a4rot,
        "zr4": np.zeros((T, hid_c.shape[1]), dtype=np.float32),
        "arb": arb,
        "sbc2": sbc2,
    }


_NC_CACHE = {}


def _get_nc(carry1=True):
    key = (L, DC, carry1)
    if key not in _NC_CACHE:
        _NC_CACHE[key] = build_nc(L, DC, carry1=carry1)
    return _NC_CACHE[key]


def run_cores(hidden_states, boundary_mask, boundary_prob, trace=False, **kw):
    """Shard, run on 8 NeuronCores, reassemble. Returns (out, BassKernelResults)."""
    from concourse.bass_utils import run_bass_kernel_spmd

    hidden_states = np.asarray(hidden_states, dtype=np.float32)
    boundary_mask = np.asarray(boundary_mask)
    boundary_prob = np.asarray(boundary_prob, dtype=np.float32)
    assert hidden_states.shape == (B, L, D)

    # 1-tile-carry specialization is exact when every tile's total decay
    # underflows f32 (true for generic random p); guard on the actual data.
    p_all = np.clip(boundary_prob[..., 1].astype(np.float64), EPS, 1.0 - EPS)
    p_all[:, 0] = 1.0
    la = np.log1p(-p_all).reshape(B, L // T, T)
    max_decay = float(np.exp(la.sum(-1)).max())
    nc = _get_nc(carry1=(max_decay < 1e-12))
    in_maps = []
    for c in range(N_CORES):
        b, dh = c // 2, c % 2
        in_maps.append(
            make_core_inputs(
                hidden_states[b, :, dh * DC : (dh + 1) * DC],
                boundary_prob[b, :, 1],
                boundary_mask[b].astype(np.float64),
            )
        )
    res = run_bass_kernel_spmd(nc, in_maps, list(range(N_CORES)), trace=trace, **kw)
    out = np.empty((B, L, D), dtype=np.float32)
    for c in range(N_CORES):
        b, dh = c // 2, c % 2
        out[b, :, dh * DC : (dh + 1) * DC] = unpermute_out(res.results[c]["out"])
    return out, res


def kernel(hidden_states, boundary_mask, boundary_prob):
    out, _ = run_cores(hidden_states, boundary_mask, boundary_prob, trace=False)
    return out


# revision 24
# speedup vs baseline: 1.1491x; 1.1491x over previous
"""DeChunk EMA-scan kernel for Trainium2 (Bass/Tile), 8 NeuronCores. V2.

Problem: out[b,t,:] = p_t * x_t + (1-p_t) * out[b,t-1,:], where
x_t = hidden[b, idx_t, :], idx = cumsum(boundary_mask)-1,
p = clip(boundary_prob[...,1], EPS, 1-EPS) with p[:,0]=1.

Sharding: pure data parallel. core c handles batch b=c//2 and channel half
dh=c%2 (512 of 1024 channels). No cross-core communication.

V2 design (vs V1 baseline in kernel_baseline.py):
  - All data-dependent-but-cheap prep is HOST-side: gather indices (int16
    wrapped layout), tile-local log-decay cumsum S (rotated + flattened),
    exp(S) carry-weight rows, per-partition exp biases, level-2 carry
    weights. Device does no cumsums -> gathers start ~8us earlier.
  - Per-tile big matmuls are PURE LOCAL scans (no carry input) and fully
    independent -> PE streams back-to-back and ramps out of the low p-state.
  - Tile carries via a LEVEL-2 blocked scan: per-tile last-row r_k collected
    at the legal partition starts 0/32/64/96 of an R4 tile (4 tiles/block);
    one K=128 matmul + one K=1 carry-in matmul per block produce the 4
    carries (block-rotated so the block carry lands at partition 0). Serial
    chain cost ~0.3us/tile instead of ~0.9 (DVE recurrence) or ~1.4 (fold).
  - Carry applied per tile by a K=1 fixup matmul accumulating into the same
    PSUM bank (exp(S) row (x) carry).
  - S-broadcast matmul in PLAIN f32 (f32r there cost 2.7e-2 rel err: S~90
    loses mantissa and exp amplifies it). Data matmuls stay f32r.
  - Output stores on the Sync HWDGE queue; GpSimd runs only the gather.
"""

import sys

for _p in ("/opt/trn_rl_repo", "/root/.axon_site/_ro/trn_rl_repo"):
    if _p not in sys.path:
        sys.path.insert(0, _p)

import numpy as np
from contextlib import ExitStack

import concourse.bass as bass
import concourse.tile as tile
from concourse import bacc, mybir
from concourse._compat import with_exitstack

B, L, D = 4, 8192, 1024
N_CORES = 8
DC = D // 2  # channels per core
T = 128  # scan tile length
BLK = 4  # tiles per level-2 carry block
SCH = 4  # tiles per S-broadcast chunk
STG = 4  # output tiles per store DMA
EPS = 1e-4
STORE_SYNC = False
F32 = mybir.dt.float32
F32R = mybir.dt.float32r
I16 = mybir.dt.int16
ALU = mybir.AluOpType
ACTF = mybir.ActivationFunctionType


@with_exitstack
def _dechunk_v2(
    ctx: ExitStack,
    tc: "tile.TileContext",
    out_ap: bass.AP,
    hid_ap: bass.AP,
    idx16_ap: bass.AP,
    sflat_ap: bass.AP,
    arow_ap: bass.AP,
    biasv_ap: bass.AP,
    mbias_ap: bass.AP,
    ones_ap: bass.AP,
    wlv2_ap: bass.AP,
    a4rot_ap: bass.AP,
    zr4_ap: bass.AP,
    arb_ap: bass.AP,
    sbc2_ap: bass.AP,
    Lk: int,
    Dk: int,
    carry1: bool = True,
):
    nc = tc.nc
    nt = Lk // T
    ns = Lk // 16
    assert nt % BLK == 0 and nt % SCH == 0 and nt % STG == 0
    nb = nt // BLK

    const = ctx.enter_context(tc.tile_pool(name="const", bufs=1))
    gat_pool = ctx.enter_context(tc.tile_pool(name="gat", bufs=10))
    lhs_pool = ctx.enter_context(tc.tile_pool(name="lhs", bufs=4))
    r4_pool = ctx.enter_context(tc.tile_pool(name="r4", bufs=2))
    c4sb_pool = ctx.enter_context(tc.tile_pool(name="c4sb", bufs=3))
    cflat_pool = ctx.enter_context(tc.tile_pool(name="cflat", bufs=2))
    outsb_pool = ctx.enter_context(tc.tile_pool(name="outsb", bufs=3))
    psum_ops = ctx.enter_context(
        tc.tile_pool(name="psum_ops", bufs=8 if carry1 else 6, space="PSUM")
    )
    psum_scol = None
    if not carry1:
        psum_scol = ctx.enter_context(
            tc.tile_pool(name="psum_scol", bufs=2, space="PSUM")
        )

    # ---- constant / host-precomputed loads. idx16 first (gathers wait on it);
    # the lhsT-path constants go on the scalar HWDGE queue in parallel.
    idx16 = const.tile([T, ns], I16)
    nc.sync.dma_start(out=idx16[:], in_=idx16_ap)
    arb = sbc2 = None
    ones_r = sflat = cmb = biasv = arow = wlv2 = a4rot = None
    if carry1:
        # sbc2: host-prebuilt EXP input (S broadcast + causal mask + bias);
        # arb: fixup lhsT bank (row 32*(k%4) of column-block k = exp(S)-row
        # k, zeros elsewhere) -> fixups are K=128 matmuls, same shape as the
        # big matmuls (shape-uniform PE work keeps the clock ramped).
        # Interleave the two big loads in pieces so early tiles start fast.
        sbc2 = const.tile([T, Lk], F32)
        arb = const.tile([T, Lk], F32R)
        npc = 8
        for i in range(npc):
            s0, s1 = i * (Lk // npc), (i + 1) * (Lk // npc)
            nc.scalar.dma_start(out=sbc2[:, s0:s1], in_=sbc2_ap[:, s0:s1])
            nc.scalar.dma_start(out=arb[:, s0:s1], in_=arb_ap[:, s0:s1])
    else:
        ones_r = const.tile([2, T], F32R)
        nc.scalar.dma_start(out=ones_r[:], in_=ones_ap)
        sflat = const.tile([2, Lk], F32R)
        nc.scalar.dma_start(out=sflat[:], in_=sflat_ap)
        cmb = const.tile([T, T], F32)
        nc.scalar.dma_start(out=cmb[:], in_=mbias_ap)
        biasv = const.tile([T, nt], F32)
        nc.scalar.dma_start(out=biasv[:], in_=biasv_ap)
        arow = const.tile([1, Lk], F32R)
        nc.scalar.dma_start(out=arow[:], in_=arow_ap)
        wlv2 = const.tile([T, nt], F32R)
        nc.scalar.dma_start(out=wlv2[:], in_=wlv2_ap)
        a4rot = const.tile([1, nt], F32R)
        nc.scalar.dma_start(out=a4rot[:], in_=a4rot_ap)

    # R4 collection tiles: rows 32j hold r_{4b+j}; all other partitions must
    # be exactly 0.0 (they stream through the level-2 matmul with zero
    # weights, and garbage NaN/Inf would poison 0*x).
    r4_tiles = [
        r4_pool.tile([T, Dk], F32R, tag="r4", name=f"r4_{i}") for i in range(2)
    ]
    for t4 in r4_tiles:
        nc.scalar.dma_start(out=t4[:], in_=zr4_ap)

    # ---- gather emission (SWDGE q1), just-in-time with small leading chunks
    chunk_tiles = []
    for sz in (1, 1, 2, 4):
        if sum(chunk_tiles) + sz <= nt:
            chunk_tiles.append(sz)
    while sum(chunk_tiles) < nt:
        chunk_tiles.append(min(4, nt - sum(chunk_tiles)))
    chunk_start = [sum(chunk_tiles[:i]) for i in range(len(chunk_tiles))]
    tile2chunk = {}
    for c, (st, sz) in enumerate(zip(chunk_start, chunk_tiles)):
        for jj in range(sz):
            tile2chunk[st + jj] = c

    gat_tiles = {}

    def emit_gather(c):
        if c >= len(chunk_tiles):
            return
        n_idx = chunk_tiles[c] * T
        g_t = gat_pool.tile([T, chunk_tiles[c] * Dk], F32R, tag="gat", name=f"gat_{c}")
        g3 = g_t[:].rearrange("p (j d) -> p j d", d=Dk)
        s0 = chunk_start[c] * T // 16
        nc.gpsimd.dma_gather(
            out_ap=g3,
            in_ap=hid_ap.bitcast(F32R),
            idxs_ap=idx16[:, s0 : s0 + n_idx // 16],
            num_idxs=n_idx,
            num_idxs_reg=n_idx,
            elem_size=Dk,
            queue_num=1 if nc.num_swdge_queues > 1 else 0,
        )
        gat_tiles[c] = g_t

    GA_TILES = 40
    emitted_chunks = 0
    emitted_tiles = 0

    def advance_gathers(k):
        nonlocal emitted_chunks, emitted_tiles
        while emitted_tiles < min(k + GA_TILES, nt):
            emit_gather(emitted_chunks)
            emitted_tiles += chunk_tiles[emitted_chunks]
            emitted_chunks += 1

    advance_gathers(0)

    # ---- main loop
    # Output rows ROTATED by one: out partition t' holds position (t'-1)%T,
    # so each tile's last position (the carry row r_k) is at partition 0.
    ops_tiles = {}
    c4sb_tiles = {}
    cflat_tiles = {}
    osb_tiles = {}

    scol_tiles = {}

    def emit_scol(c):
        # S-broadcast chunk for tiles 4c..4c+3, emitted ~2 tiles early so the
        # mask-add + EXP chain never stalls the first big matmul of a block
        if c >= nt // SCH:
            return
        tmp_ps = psum_scol.tile([T, SCH * T], F32, tag="scol")
        nc.tensor.matmul(
            tmp_ps[:],
            lhsT=ones_r[:],
            rhs=sflat[:, c * SCH * T : (c + 1) * SCH * T],
            start=True,
            stop=True,
        )
        scol_tiles[c] = tmp_ps

    def emit_front(k):
        # lhsT build + pure-local big matmul for tile k
        lhsT_k = lhs_pool.tile([T, T], F32R, tag="lhsT")
        if carry1:
            nc.scalar.activation(
                lhsT_k[:], sbc2[:, k * T : (k + 1) * T], ACTF.Exp
            )
        else:
            tmp_ps = scol_tiles[k // SCH]
            j = k % SCH
            nc.vector.tensor_tensor(
                out=tmp_ps[:, j * T : (j + 1) * T],
                in0=tmp_ps[:, j * T : (j + 1) * T],
                in1=cmb[:],
                op=ALU.add,
            )
            nc.scalar.activation(
                lhsT_k[:],
                tmp_ps[:, j * T : (j + 1) * T],
                ACTF.Exp,
                bias=biasv[:, k : k + 1],
                scale=1.0,
            )
            if j == SCH - 1:
                scol_tiles.pop(k // SCH)
        ops = psum_ops.tile([T, Dk], F32, tag="ops")
        cg = tile2chunk[k]
        rhs = gat_tiles[cg][:].rearrange("p (j d) -> p j d", d=Dk)[
            :, k - chunk_start[cg], :
        ]
        nc.tensor.matmul(ops[:], lhsT=lhsT_k[:], rhs=rhs, start=True, stop=True)
        ops_tiles[k] = ops
        # collect r_k = local scan last row (rotated to partition 0) into the
        # SHIFTED window tile: consumer block i = (k+1)//BLK reads r rows of
        # tiles 4i-1..4i+2 at partitions 32*((k+1)%BLK)
        if k < nt - 1:
            i = (k + 1) // BLK
            jb = (k + 1) % BLK
            r4 = r4_tiles[i % 2]
            if k % 2 == 0:
                nc.scalar.copy(r4[32 * jb : 32 * jb + 1, :], ops[0:1, :])
            else:
                nc.vector.tensor_copy(r4[32 * jb : 32 * jb + 1, :], ops[0:1, :])

    def emit_block_carries(b):
        # level-2 over the SHIFTED window (tiles 4b-1..4b+2): produces the
        # carries needed by fixups of tiles 4b..4b+3 one tile early, so the
        # c4sb-copy + cflat-DMA latency is fully hidden. Block-rotated:
        # C4 row 0 = c_{4b+2}; rows 1..3 = c_{4b-1}, c_{4b}, c_{4b+1}.
        r4 = r4_tiles[b % 2]
        c4_ps = psum_scol.tile([BLK, Dk], F32, tag="scol")
        nc.tensor.matmul(
            c4_ps[:],
            lhsT=wlv2[:, b * BLK : (b + 1) * BLK],
            rhs=r4[:],
            start=True,
            stop=(b == 0),
        )
        if b > 0:
            nc.tensor.matmul(
                c4_ps[:],
                lhsT=a4rot[0:1, b * BLK : (b + 1) * BLK],
                rhs=c4sb_tiles[b - 1][0:1, :],
                start=False,
                stop=True,
            )
        c4sb = c4sb_pool.tile([BLK, Dk], F32R, tag="c4sb", name=f"c4sb_{b}")
        nc.vector.tensor_copy(c4sb[:], c4_ps[:])
        c4sb_tiles[b] = c4sb
        # rows 1..3 (c_{4b}..c_{4b+2}) flattened to partition 0 for the
        # within-block fixup matmul rhs reads
        cflat = cflat_pool.tile([1, (BLK - 1) * Dk], F32R, tag="cflat", name=f"cf_{b}")
        nc.sync.dma_start(
            out=cflat[:].rearrange("p (j d) -> p j d", d=Dk), in_=c4sb[1:BLK, :]
        )
        cflat_tiles[b] = cflat
        if b >= 2:
            c4sb_tiles.pop(b - 2)
            cflat_tiles.pop(b - 2)

    def emit_fixup(k):
        # ops_k += exp(S)-row (x) c_{k-1}, accumulated into the same bank
        if k == 0:
            return
        b, jb = k // BLK, k % BLK
        if carry1:
            # c_{k-1} = r_{k-1} exactly (inter-tile decay underflows f32).
            # K=128 form: lhsT = ARB column block (zeros except row 32*jb
            # which holds exp(S)-row k); rhs = whole r4 window tile.
            nc.tensor.matmul(
                ops_tiles[k][:],
                lhsT=arb[:, k * T : (k + 1) * T],
                rhs=r4_tiles[(k // BLK) % 2][:],
                start=False,
                stop=True,
                skip_group_check=True,
            )
            return
        elif jb == BLK - 1:
            c_prev = c4sb_tiles[b][0:1, :]
        else:
            c_prev = cflat_tiles[b][0:1, jb * Dk : (jb + 1) * Dk]
        nc.tensor.matmul(
            ops_tiles[k][:],
            lhsT=arow[0:1, k * T : (k + 1) * T],
            rhs=c_prev,
            start=False,
            stop=True,
            skip_group_check=True,
        )

    def emit_back(k):
        # final PSUM -> SBUF staging + batched store (sync HWDGE)
        ops = ops_tiles.pop(k)
        if k % STG == 0:
            osb_tiles[k // STG] = outsb_pool.tile(
                [T, STG * Dk], F32, tag="osb", name=f"osb_{k // STG}"
            )
        osb = osb_tiles[k // STG]
        dst = osb[:, (k % STG) * Dk : (k % STG + 1) * Dk]
        if k % 2 == 0:
            nc.scalar.copy(dst, ops[:])
        else:
            nc.vector.tensor_copy(dst, ops[:])
        if k % STG == STG - 1:
            g0 = k - (STG - 1)
            store_eng = nc.sync if STORE_SYNC else nc.gpsimd
            store_eng.dma_start(
                out=out_ap.rearrange("p (k d) -> p k d", d=Dk)[:, g0 : g0 + STG, :],
                in_=osb_tiles.pop(k // STG)[:].rearrange("p (k d) -> p k d", d=Dk),
            )

    # Fixups/stores for block b-1 are emitted while block b's big matmuls
    # stream, so the PE never waits on the c4sb copy + cflat DMA latency
    # (gapless PE keeps the p-state ramp alive).
    if not carry1:
        emit_scol(0)
    if carry1:
        # fixups/stores for block b-1 batched at j==1 of block b: all their
        # r-copies are >=2 tiles old (no stalls) and the matmul stream stays
        # shape-uniform for the PE clock ramp
        for k in range(nt):
            advance_gathers(k)
            b, jb = k // BLK, k % BLK
            if jb == 1 and b >= 1:
                for kk in range((b - 1) * BLK, b * BLK):
                    emit_fixup(kk)
                for kk in range((b - 1) * BLK, b * BLK):
                    emit_back(kk)
            emit_front(k)
        for kk in range(nt - BLK, nt):
            emit_fixup(kk)
        for kk in range(nt - BLK, nt):
            emit_back(kk)
    else:
        for k in range(nt):
            advance_gathers(k)
            b, jb = k // BLK, k % BLK
            if jb == 1 and b >= 1:
                for kk in range((b - 1) * BLK, b * BLK):
                    emit_fixup(kk)
                for kk in range((b - 1) * BLK, b * BLK):
                    emit_back(kk)
            if jb == 2:
                emit_scol(b + 1)
            emit_front(k)
            if jb == BLK - 1:
                emit_block_carries(b)
        for kk in range((nb - 1) * BLK, nt):
            emit_fixup(kk)
        for kk in range((nb - 1) * BLK, nt):
            emit_back(kk)


def _host_constants():
    s = np.arange(T)[:, None]
    t = np.arange(T)[None, :]
    t_rot = (t - 1) % T
    mbias = np.where(s <= t_rot, 0.0, -3e38).astype(np.float32)
    ones = np.ones((1, T), dtype=np.float32)
    return mbias, ones


def build_nc(Lk=L, Dk=DC, carry1=True):
    nt = Lk // T
    nc = bacc.Bacc(
        "TRN2",
        target_bir_lowering=False,
        debug=False,
        enable_asserts=False,
        num_swdge_queues=2,
    )
    hid = nc.dram_tensor("hid", [Lk, Dk], F32, kind="ExternalInput").ap()
    idx16 = nc.dram_tensor("idx16", [T, Lk // 16], I16, kind="ExternalInput").ap()
    sflat = nc.dram_tensor("sflat", [2, Lk], F32R, kind="ExternalInput").ap()
    arow = nc.dram_tensor("arow", [1, Lk], F32R, kind="ExternalInput").ap()
    biasv = nc.dram_tensor("biasv", [T, nt], F32, kind="ExternalInput").ap()
    mbias = nc.dram_tensor("mbias", [T, T], F32, kind="ExternalInput").ap()
    ones = nc.dram_tensor("ones", [2, T], F32R, kind="ExternalInput").ap()
    wlv2 = nc.dram_tensor("wlv2", [T, nt], F32R, kind="ExternalInput").ap()
    a4rot = nc.dram_tensor("a4rot", [1, nt], F32R, kind="ExternalInput").ap()
    zr4 = nc.dram_tensor("zr4", [T, Dk], F32R, kind="ExternalInput").ap()
    arb = nc.dram_tensor("arb", [T, Lk], F32R, kind="ExternalInput").ap()
    sbc2 = nc.dram_tensor("sbc2", [T, Lk], F32, kind="ExternalInput").ap()
    # raw partition-major layout: out[p, k*Dk + d] = y[k*T + (p-1)%T, d]
    out = nc.dram_tensor("out", [T, nt * Dk], F32, kind="ExternalOutput").ap()
    with tile.TileContext(nc) as tc:
        _dechunk_v2(
            tc, out, hid, idx16, sflat, arow, biasv, mbias, ones, wlv2, a4rot, zr4, arb, sbc2, Lk, Dk,
            carry1=carry1,
        )
    nc.compile()
    return nc


def unpermute_out(raw, Lk=L, Dk=DC):
    """raw (T, nt*Dk) partition-major rotated -> (Lk, Dk) sequence order."""
    nt = Lk // T
    raw = raw.reshape(T, nt, Dk)
    rr = raw[(np.arange(T) + 1) % T]  # rr[q, k] = y[k*T + q]
    return np.ascontiguousarray(rr.transpose(1, 0, 2).reshape(Lk, Dk))


def make_core_inputs(hid_c, p_c, m_c, Lk=L):
    """Host-side prep. hid_c (Lk, Dk) f32; p_c (Lk,) raw probs; m_c (Lk,) mask."""
    nt = Lk // T
    idx = np.cumsum(np.asarray(m_c, dtype=np.int64)) - 1
    idxw = idx.astype(np.int16).reshape(Lk // 16, 16).T  # wrapped [16, ns]
    idx16 = np.ascontiguousarray(np.tile(idxw, (8, 1)))  # [128, ns]

    p = np.clip(np.asarray(p_c, dtype=np.float64), EPS, 1.0 - EPS)
    p[0] = 1.0
    a = 1.0 - p
    a[0] = 1.0  # cancels in all weights; position 0 has no carry
    loga = np.log(a).reshape(nt, T)
    S = np.cumsum(loga, axis=1)  # tile-local inclusive cumsum [nt, T]
    biasv = (np.log(p).reshape(nt, T) - S).T.astype(np.float32)  # [T, nt]
    rot = (np.arange(T) - 1) % T
    srot = S[:, rot].reshape(Lk)  # srot[k*T+t'] = S[k, (t'-1)%T]
    arow = np.exp(srot).astype(np.float32).reshape(1, Lk)
    # coarse/fine split so the K=2 f32r broadcast matmul reconstructs S
    # exactly: coarse keeps 10 mantissa bits (f32r-exact for any >=10-bit
    # format); fine = S - coarse is ~2^-10 smaller, its own rounding error
    # is negligible after exp().
    sc = np.ascontiguousarray(srot.astype(np.float32))
    sc_i = sc.view(np.uint32)
    sc_i &= np.uint32(0xFFFFE000)  # keep 10 explicit mantissa bits
    fine = (srot - sc.astype(np.float64)).astype(np.float32)
    sflat = np.stack([sc, fine]).reshape(2, Lk)

    # level-2 carry weights: c_k = A_k c_{k-1} + r_k, A_k = exp(S[k, -1]).
    # Block b covers tiles k0..k0+3; C4 row j' holds c_{k0+(j'+3)%4}.
    A = np.exp(S[:, -1])  # [nt]
    # Shifted level-2 windows: block b combines r of tiles 4b-1..4b+2 (slot m
    # holds tile 4b-1+m; slot 0 of block 0 is the zero r_{-1} row). Rotated
    # columns: j'=0 -> c_{4b+2}, j'=1..3 -> c_{4b-1..4b+1}.
    wlv2 = np.zeros((T, nt), dtype=np.float32)
    a4rot = np.zeros((1, nt), dtype=np.float32)
    for b in range(nt // BLK):
        for jp in range(BLK):
            jj = (jp + BLK - 1) % BLK
            acc = 1.0
            for m in range(jj, -1, -1):
                w = b * BLK - 1 + m  # global tile index of window slot m
                if w >= 0:
                    # weight of r_{w} into c_{4b-1+jj}
                    wlv2[32 * m, b * BLK + jp] = np.float32(acc)
                    acc *= A[w]
                else:
                    acc = 0.0  # r_{-1}: no contribution, no carry-in
            a4rot[0, b * BLK + jp] = np.float32(acc)
    mbias, ones = _host_constants()
    ones = np.ones((2, T), dtype=np.float32)
    arb = np.zeros((T, Lk), dtype=np.float32)
    arow_f = arow.reshape(nt, T)
    for k in range(1, nt):
        arb[32 * (k % BLK), k * T : (k + 1) * T] = arow_f[k]
    # sbc2[s, k*T+t'] = S[k,(t'-1)%T] + causal-mask(s,t') + (logp - S)[k,s]:
    # the full EXP input precomputed in f64, one Scal op per tile on device
    srot_kt = S[:, rot]  # [nt, T]
    bias_ks = np.log(p).reshape(nt, T) - S  # [nt, T]
    sbc2 = (
        srot_kt[None, :, :]
        + mbias.astype(np.float64)[:, None, :]
        + bias_ks.T[:, :, None]
    )
    sbc2 = np.ascontiguousarray(
        np.maximum(sbc2, -3e38).astype(np.float32).reshape(T, Lk)
    )
    return {
        "hid": np.ascontiguousarray(hid_c, dtype=np.float32),
        "idx16": idx16,
        "sflat": np.ascontiguousarray(sflat),
        "arow": np.ascontiguousarray(arow),
        "biasv": np.ascontiguousarray(biasv),
        "mbias": mbias,
        "ones": ones,
        "wlv2": wlv2,
        "a4rot": a4rot,
        "zr4": np.zeros((T, hid_c.shape[1]), dtype=np.float32),
        "arb": arb,
        "sbc2": sbc2,
    }


_NC_CACHE = {}


def _get_nc(carry1=True):
    key = (L, DC, carry1)
    if key not in _NC_CACHE:
        _NC_CACHE[key] = build_nc(L, DC, carry1=carry1)
    return _NC_CACHE[key]


def run_cores(hidden_states, boundary_mask, boundary_prob, trace=False, **kw):
    """Shard, run on 8 NeuronCores, reassemble. Returns (out, BassKernelResults)."""
    from concourse.bass_utils import run_bass_kernel_spmd

    hidden_states = np.asarray(hidden_states, dtype=np.float32)
    boundary_mask = np.asarray(boundary_mask)
    boundary_prob = np.asarray(boundary_prob, dtype=np.float32)
    assert hidden_states.shape == (B, L, D)

    # 1-tile-carry specialization is exact when every tile's total decay
    # underflows f32 (true for generic random p); guard on the actual data.
    p_all = np.clip(boundary_prob[..., 1].astype(np.float64), EPS, 1.0 - EPS)
    p_all[:, 0] = 1.0
    la = np.log1p(-p_all).reshape(B, L // T, T)
    max_decay = float(np.exp(la.sum(-1)).max())
    nc = _get_nc(carry1=(max_decay < 1e-12))
    in_maps = []
    for c in range(N_CORES):
        b, dh = c // 2, c % 2
        in_maps.append(
            make_core_inputs(
                hidden_states[b, :, dh * DC : (dh + 1) * DC],
                boundary_prob[b, :, 1],
                boundary_mask[b].astype(np.float64),
            )
        )
    res = run_bass_kernel_spmd(nc, in_maps, list(range(N_CORES)), trace=trace, **kw)
    out = np.empty((B, L, D), dtype=np.float32)
    for c in range(N_CORES):
        b, dh = c // 2, c % 2
        out[b, :, dh * DC : (dh + 1) * DC] = unpermute_out(res.results[c]["out"])
    return out, res


def kernel(hidden_states, boundary_mask, boundary_prob):
    out, _ = run_cores(hidden_states, boundary_mask, boundary_prob, trace=False)
    return out
